# revision 2
# baseline (speedup 1.0000x reference)
"""Trainium2 Bass kernel for the CustomGRU problem.

Reference semantics (fp32):
    z = sigmoid(x_t @ Wz_x + bz + h @ Wz_h)
    r = sigmoid(x_t @ Wr_x + br + h @ Wr_h)
    h~ = tanh(x_t @ Wh_x + bh + (r*h) @ Wh_h)
    h  = (1-z)*h + z*h~            (T=512 steps)
    out = h_T @ Wfc + bfc

Sharding: pure data parallel over batch (8192 -> 8 cores x 1024); the
time recurrence runs locally per core; the tiny weights are replicated.

Per-core design (H-major layout, B=1024 split into G=2 independently
pipelined batch groups of 512 so consecutive steps overlap across
engines):
  - state h lives in SBUF tiles [33, Bg] at partition base 0
  - gate matmuls are accumulate-split: x-part (K=8, bf16, streamed from
    wide staging tiles holding 4 steps, one DMA each) + h-part (K=33,
    fp32). The x-part of step t uses PE row strip 32*(t%4) with its own
    copy of the x-weights at those partitions (lhsT and rhs must share
    base partitions).
  - r -> psum rows 0-32 (PE cols 0-63), z -> psum rows 64-96
    (tile_position col 64) so every tensor_tensor operand pair shares a
    32-aligned base partition (HW: both DVE inputs need equal bases; a
    33-row access cannot start at 32 or 96).
  - one sigmoid ACT call reads psum rows 0..96 (rows 33-63 zeroed once
    in persistent ping-pong psum tiles); gate biases ride the ACT
    per-partition bias operand.
"""

import sys

sys.path.insert(0, "/opt/trn_rl_repo")

from contextlib import ExitStack

import ml_dtypes  # noqa: F401  (registers bfloat16 with numpy)
import numpy as np
import orjson

import concourse.bacc as bacc
import concourse.bass as bass
import concourse.tile as tile
from concourse import mybir
from concourse.bass_utils import run_bass_kernel_spmd

N_CORES = 8
I_IN = 8
H = 33
HOR = 24

AF = mybir.ActivationFunctionType
DT = mybir.dt
BF16 = np.dtype("bfloat16")


# --------------------------------------------------------------------------
# walrus in this container rejects CTRL (Drain) instructions carrying more
# than one sync wait; Tile's kernel-tail drain always has several. Split
# them at the serialized-JSON level (mutating the live module corrupts it).
def _split_multiwait_drains(raw: bytes, max_waits: int = 1) -> bytes:
    m = orjson.loads(raw)
    changed = False
    for f in m["functions"]:
        for bb in f["blocks"]:
            out = []
            for inst in bb["instructions"]:
                si = inst.get("sync_info")
                ow = (si or {}).get("on_wait") or []
                if inst.get("opcode") == "Drain" and len(ow) > max_waits:
                    head, tail = ow[:-max_waits], ow[-max_waits:]
                    for k, w in enumerate(head):
                        clone = dict(inst)
                        clone["name"] = f"{inst['name']}-sw{k}"
                        clone["sync_info"] = {"on_update": [], "on_wait": [w]}
                        out.append(clone)
                    inst = dict(inst)
                    inst["sync_info"] = {
                        "on_update": si.get("on_update") or [],
                        "on_wait": tail,
                    }
                    changed = True
                out.append(inst)
            bb["instructions"] = out
    return orjson.dumps(m) if changed else raw


def _install_bir_patch(nc):
    orig = nc.to_json_bytes
    nc.to_json_bytes = lambda: _split_multiwait_drains(orig())


# --------------------------------------------------------------------------
XSTEPS = 4  # x row strips per staging tile (strips 0/32/64/96, rows +0..7)
NB = 16  # steps per strip per staging tile; one tile covers XSTEPS*NB steps
XBLK = XSTEPS * NB


def build_gru_nc(B: int, T: int, finalize: bool = True, G: int = 2, repeat: int = 1,
                 elem16: bool = True):
    """Build the per-core Bass module (B = per-core batch). repeat>1 runs the
    whole recurrence multiple times (for wall-clock delta timing)."""
    nc = bacc.Bacc("TRN2", target_bir_lowering=False, debug=False)
    f32 = DT.float32
    bf16 = DT.bfloat16
    edt = DT.float16 if elem16 else DT.float32
    Bg = B // G
    assert T % XBLK == 0 and B % G == 0

    # host layout: xH[blk, j, i, k, b] = x[b, blk*XBLK + k*XSTEPS + j, i]
    xH = nc.dram_tensor(
        "xH", [T // XBLK, XSTEPS, I_IN, NB, B], bf16, kind="ExternalInput"
    ).ap()
    w_r_h = nc.dram_tensor("w_r_h", [H, H], edt, kind="ExternalInput").ap()
    w_z_h = nc.dram_tensor("w_z_h", [H, H], edt, kind="ExternalInput").ap()
    w_h_h = nc.dram_tensor("w_h_h", [H, H], edt, kind="ExternalInput").ap()
    w_r_x = nc.dram_tensor("w_r_x", [I_IN, H], bf16, kind="ExternalInput").ap()
    w_z_x = nc.dram_tensor("w_z_x", [I_IN, H], bf16, kind="ExternalInput").ap()
    w_h_x = nc.dram_tensor("w_h_x", [I_IN, H], bf16, kind="ExternalInput").ap()
    b_sig = nc.dram_tensor("b_sig", [97, 1], f32, kind="ExternalInput").ap()
    b_h = nc.dram_tensor("b_h", [H, 1], f32, kind="ExternalInput").ap()
    w_fc = nc.dram_tensor("w_fc", [H, HOR], edt, kind="ExternalInput").ap()
    b_fc = nc.dram_tensor("b_fc", [HOR, 1], f32, kind="ExternalInput").ap()
    y = nc.dram_tensor("y", [HOR, B], f32, kind="ExternalOutput").ap()

    with tile.TileContext(nc) as tc:
        with ExitStack() as ctx:
            consts = ctx.enter_context(tc.tile_pool(name="consts", bufs=1))
            hpool = ctx.enter_context(tc.tile_pool(name="hpool", bufs=6))
            rhpool = ctx.enter_context(tc.tile_pool(name="rhpool", bufs=6))
            acts = ctx.enter_context(tc.tile_pool(name="acts", bufs=6))
            xstage = ctx.enter_context(tc.tile_pool(name="xstage", bufs=2))
            psum_zr = ctx.enter_context(
                tc.tile_pool(name="psum_zr", bufs=1, space="PSUM")
            )
            psum_h = ctx.enter_context(
                tc.tile_pool(name="psum_h", bufs=2, space="PSUM")
            )

            # ---- constants ----
            wrh_t = consts.tile([H, H], edt)
            wzh_t = consts.tile([H, H], edt)
            whh_t = consts.tile([H, H], edt)
            # x-weights: one copy per PE row strip (rows 32j..32j+7)
            wx_t = consts.tile([128, 3 * H], bf16)  # cols: [r | z | h] per strip
            bsig_t = consts.tile([97, 1], f32)
            bh_t = consts.tile([H, 1], f32)
            wfc_t = consts.tile([H, HOR], edt)
            bfc_t = consts.tile([HOR, 1], f32)
            for tl, src in [
                (wrh_t, w_r_h),
                (wzh_t, w_z_h),
                (whh_t, w_h_h),
                (bsig_t, b_sig),
                (bh_t, b_h),
                (wfc_t, w_fc),
                (bfc_t, b_fc),
            ]:
                nc.sync.dma_start(tl[:], src[:])
            for j in range(XSTEPS):
                r0 = 32 * j
                nc.sync.dma_start(wx_t[r0 : r0 + I_IN, 0:H], w_r_x[:])
                nc.sync.dma_start(wx_t[r0 : r0 + I_IN, H : 2 * H], w_z_x[:])
                nc.sync.dma_start(wx_t[r0 : r0 + I_IN, 2 * H : 3 * H], w_h_x[:])

            # ---- per-group state ----
            h_cur = []
            for g in range(G):
                h0 = hpool.tile([H, Bg], edt, tag=f"h{g}", name=f"h{g}_init")
                nc.vector.memset(h0[:, :], 0.0)
                h_cur.append(h0)

            pzr = [
                [
                    psum_zr.tile(
                        [97, Bg], f32, tag=f"pzr{g}_{i}", name=f"pzr{g}_{i}"
                    )
                    for i in range(2)
                ]
                for g in range(G)
            ]
            for g in range(G):
                for pb in pzr[g]:
                    # rows 33-63 are never written by the gate matmuls but the
                    # [97,*] sigmoid reads them; zero once (32-aligned access,
                    # row 32 is re-written by the r matmuls afterwards)
                    nc.vector.memset(pb[32:64, :], 0.0)

            xs_cur = [None] * G
            sig_cur = [None] * G
            q_cur = [None] * G

            def emit_x(g, t, rep):
                """x DMA (block granularity) + x-part matmuls (no h dep):
                opens the psum accumulation groups one step early so the
                recurrence-critical h-part matmuls start without waiting."""
                j = t % XSTEPS
                if t % XBLK == 0:
                    blk = t // XBLK
                    xs = xstage.tile(
                        [128, NB * Bg], bf16, tag=f"xs{g}", name=f"xs{g}_{rep}_{t}"
                    )
                    for jj in range(XSTEPS):
                        dst = xs[32 * jj : 32 * jj + I_IN, :].rearrange(
                            "p (k b) -> p k b", b=Bg
                        )
                        src = xH[blk, jj, :, :, g * Bg : (g + 1) * Bg]
                        nc.sync.dma_start(dst, src)
                    xs_cur[g] = xs
                xs = xs_cur[g]
                r0 = 32 * j
                k = (t // XSTEPS) % NB
                xrhs = xs[r0 : r0 + I_IN, k * Bg : (k + 1) * Bg]
                P = pzr[g][t % 2]
                PH = psum_h.tile([H, Bg], f32, tag=f"ph{g}", name=f"ph{g}_{rep}_{t}")
                nc.tensor.matmul(
                    P[0:H, :], wx_t[r0 : r0 + I_IN, 0:H], xrhs,
                    start=True, stop=False, tile_position=(r0, 0),
                )
                # the r/z/h accumulation groups live in the same psum bank;
                # per-element has_written bits make concurrent groups safe
                nc.tensor.matmul(
                    P[64 : 64 + H, :], wx_t[r0 : r0 + I_IN, H : 2 * H], xrhs,
                    start=True, stop=False, tile_position=(r0, 64),
                    skip_group_check=True,
                )
                nc.tensor.matmul(
                    PH[:, :], wx_t[r0 : r0 + I_IN, 2 * H : 3 * H], xrhs,
                    start=True, stop=False, tile_position=(r0, 0),
                    skip_group_check=True,
                )
                return PH

            def emit_front(g, t, rep, PH):
                """h-part gate matmuls, sigmoid, rh, MM_h h-part."""
                h = h_cur[g]
                P = pzr[g][t % 2]
                nc.tensor.matmul(
                    P[0:H, :], wrh_t[:], h[:, :],
                    start=False, stop=True, tile_position=(0, 0),
                )
                nc.tensor.matmul(
                    P[64 : 64 + H, :], wzh_t[:], h[:, :],
                    start=False, stop=True, tile_position=(0, 64),
                    skip_group_check=True,
                )
                sig = acts.tile(
                    [97, Bg], edt, tag=f"sig{g}", name=f"sig{g}_{rep}_{t}"
                )
                nc.scalar.activation(
                    sig[0:97, :], P[0:97, :], AF.Sigmoid, bias=bsig_t[:]
                )
                sig_cur[g] = sig
                # u = 1 - z  (off the recurrence-critical path, on GpSimd)
                uq = acts.tile([H, Bg], edt, tag=f"uq{g}", name=f"uq{g}_{rep}_{t}")
                nc.vector.tensor_scalar(
                    uq[:, :], sig[64 : 64 + H, :], -1.0, 1.0,
                    op0=mybir.AluOpType.mult, op1=mybir.AluOpType.add,
                )
                # q = (1-z) * h  (also off-cycle)
                q = rhpool.tile([H, Bg], edt, tag=f"q{g}", name=f"q{g}_{rep}_{t}")
                nc.vector.tensor_mul(q[:, :], uq[:, :], h[:, :])
                q_cur[g] = q
                # rh = r * h
                rh = rhpool.tile([H, Bg], edt, tag=f"rh{g}", name=f"rh{g}_{rep}_{t}")
                nc.vector.tensor_mul(rh[:, :], sig[0:H, :], h[:, :])
                nc.tensor.matmul(
                    PH[:, :], whh_t[:], rh[:, :],
                    start=False, stop=True, tile_position=(0, 0),
                    skip_group_check=True,
                )
                return PH

            def emit_back(g, t, rep, PH):
                """tanh -> p = z*h~ -> h' = p + q  (2-stage critical tail)."""
                sig = sig_cur[g]
                hts = acts.tile(
                    [97, Bg], edt, tag=f"hts{g}", name=f"hts{g}_{rep}_{t}"
                )
                # h~ lands at base 64 so the z*h~ operands share base 64
                nc.scalar.activation(
                    hts[64 : 64 + H, :], PH[:, :], AF.Tanh, bias=bh_t[:]
                )
                # p = z * h~
                nc.vector.tensor_mul(
                    hts[0:H, :], sig[64 : 64 + H, :], hts[64 : 64 + H, :]
                )
                # h' = p + q
                h_new = hpool.tile(
                    [H, Bg], edt, tag=f"h{g}", name=f"h{g}_{rep}_{t}"
                )
                nc.vector.tensor_add(h_new[:, :], hts[0:H, :], q_cur[g][:, :])
                h_cur[g] = h_new

            for rep in range(repeat):
                if rep > 0:
                    for g in range(G):
                        h0 = hpool.tile(
                            [H, Bg], edt, tag=f"h{g}", name=f"h{g}_init{rep}"
                        )
                        nc.vector.memset(h0[:, :], 0.0)
                        h_cur[g] = h0
                # software pipeline: group 1 runs half a step behind group 0
                # so the FIFO engine queues interleave front and back halves.
                ph_x = [None] * G  # PH tile of the step whose x-MMs ran
                ph_pend = [None] * G
                xq = [[None] * T for _ in range(G)]
                for g in range(G):
                    xq[g][0] = emit_x(g, 0, rep)
                for t in range(T):
                    if t + 1 < T:
                        xq[0][t + 1] = emit_x(0, t + 1, rep)
                    ph_pend[0] = emit_front(0, t, rep, xq[0][t])
                    if t > 0:
                        emit_back(1, t - 1, rep, ph_pend[1])
                    if t + 1 < T:
                        xq[1][t + 1] = emit_x(1, t + 1, rep)
                    ph_pend[1] = emit_front(1, t, rep, xq[1][t])
                    emit_back(0, t, rep, ph_pend[0])
                emit_back(1, T - 1, rep, ph_pend[1])

            # ---- final FC ----
            for g in range(G):
                pfc = psum_h.tile(
                    [HOR, Bg], f32, tag=f"ph{g}", name=f"pfc{g}"
                )
                nc.tensor.matmul(
                    pfc[:, :], wfc_t[:], h_cur[g][:, :], start=True, stop=True
                )
                y_sb = acts.tile([HOR, Bg], f32, tag=f"sig{g}", name=f"ysb{g}")
                nc.scalar.activation(
                    y_sb[0:HOR, :], pfc[:, :], AF.Identity, bias=bfc_t[:]
                )
                nc.sync.dma_start(y[:, g * Bg : (g + 1) * Bg], y_sb[0:HOR, :])

    if finalize:
        nc.finalize()
        _install_bir_patch(nc)
    return nc


def prep_weights(Wz, bz, Wr, br, Wh, bh, Wfc, bfc, elem16=True):
    ed = np.float16 if elem16 else np.float32
    b_sig = np.zeros((97, 1), np.float32)
    b_sig[0:H, 0] = br
    b_sig[64 : 64 + H, 0] = bz
    return {
        "w_r_h": np.ascontiguousarray(Wr[I_IN:]).astype(ed),
        "w_z_h": np.ascontiguousarray(Wz[I_IN:]).astype(ed),
        "w_h_h": np.ascontiguousarray(Wh[I_IN:]).astype(ed),
        "w_r_x": np.ascontiguousarray(Wr[:I_IN]).astype(BF16),
        "w_z_x": np.ascontiguousarray(Wz[:I_IN]).astype(BF16),
        "w_h_x": np.ascontiguousarray(Wh[:I_IN]).astype(BF16),
        "b_sig": b_sig,
        "b_h": np.asarray(bh).reshape(H, 1).astype(np.float32),
        "w_fc": np.ascontiguousarray(Wfc).astype(ed),
        "b_fc": np.asarray(bfc).reshape(HOR, 1).astype(np.float32),
    }


def prep_x_one_core(xc, elem16=True):
    """xc [B, T, I] -> xH host layout for one core."""
    B, T, _ = xc.shape
    xTc = xc.transpose(1, 2, 0)  # [T, I, B]
    return np.ascontiguousarray(
        xTc.reshape(T // XBLK, NB, XSTEPS, I_IN, B).transpose(0, 2, 3, 1, 4)
    ).astype(BF16)


def prep_inputs_one_core(x, Wz, bz, Wr, br, Wh, bh, Wfc, bfc, elem16=True):
    wmap = prep_weights(Wz, bz, Wr, br, Wh, bh, Wfc, bfc, elem16=elem16)
    return {"xH": prep_x_one_core(x, elem16=elem16), **wmap}


def run_gru(x, Wz, bz, Wr, br, Wh, bh, Wfc, bfc, n_cores=N_CORES, G=2,
            elem16=True, **spmd_kwargs):
    B_total, T, _ = x.shape
    B = B_total // n_cores
    nc = build_gru_nc(B, T, G=G, elem16=elem16)
    wmap = prep_weights(Wz, bz, Wr, br, Wh, bh, Wfc, bfc, elem16=elem16)
    in_maps = []
    for c in range(n_cores):
        xc = x[c * B : (c + 1) * B]  # [B, T, I]
        xTc = xc.transpose(1, 2, 0)  # [T, I, B]
        # xH[blk, j, i, k, b] = x[b, blk*XBLK + k*XSTEPS + j, i]
        xHc = np.ascontiguousarray(
            xTc.reshape(T // XBLK, NB, XSTEPS, I_IN, B).transpose(0, 2, 3, 1, 4)
        ).astype(BF16)
        in_maps.append({"xH": xHc, **wmap})
    res = run_bass_kernel_spmd(
        nc, in_maps, core_ids=list(range(n_cores)), **spmd_kwargs
    )
    y = np.concatenate(
        [res.results[c]["y"].T for c in range(n_cores)], axis=0
    ).astype(np.float32)
    return y, res


def kernel(x, Wz, bz, Wr, br, Wh, bh, Wfc, bfc):
    y, _ = run_gru(x, Wz, bz, Wr, br, Wh, bh, Wfc, bfc)
    return y



# revision 11
# speedup vs baseline: 1.4593x; 1.4593x over previous
"""Trainium2 Bass kernel for the CustomGRU problem (v3, latency-optimized).

Reference semantics (fp32):
    z = sigmoid(x_t @ Wz_x + bz + h @ Wz_h)
    r = sigmoid(x_t @ Wr_x + br + h @ Wr_h)
    h~ = tanh(x_t @ Wh_x + bh + (r*h) @ Wh_h)
    h  = (1-z)*h + z*h~            (T=512 steps)
    out = h_T @ Wfc + bfc

Sharding: pure data parallel over batch (8192 -> 8 cores x 1024); the time
recurrence runs locally per core; the tiny weights are replicated.

The kernel is bound by the per-step serial chain latency (the recurrence),
not engine throughput, so v3 minimizes the chain:

    sigma -> rh -> mm_cand -> tanh -> d -> m -> mm_gm -> sigma(t+1)

  - deferred-accumulate gates matmul: h_t = h_{t-1} + m_t, so
    gates_pre(t+1) = [W_g.[h_{t-1}; x_{t+1}]]  (start mm, runs a full cycle
    early, off the chain) + W_gh.m_t (small stop mm right after m_t). The
    h' add itself is off-chain on GpSimd/Pool.
  - z and r are M=33 matmul pairs into one [97, Bg] psum (z rows 0-32,
    r rows 64-96; rows 33-63 memset once per psum buffer) -> ONE sigmoid
    ACT per step per group, biases ride the ACT bias operand.
  - x_t rides in the matmul rhs: h tiles are [105, Bg] with h at rows
    64-96 and x rows 97-104 (overwritten with x_{t+1} after the partial
    gates mm consumed x_t); the cand matmul is one K=41 instr on the
    rh tile (rh rows 0-32 + x rows 33-40).
  - d = h~ - h and m = z*d are adjacent DVE ops (no cross-engine hop);
    all tensor_tensor ops are fp16 SBUF dense (DVE 2x mode) with operand
    partition bases equal and 32-aligned (h at base 64, d at base 0).
  - G groups of Bg=B/G samples pipeline their chains to keep engines fed.
"""

import sys

sys.path.insert(0, "/opt/trn_rl_repo")

from contextlib import ExitStack

import ml_dtypes  # noqa: F401  (registers bfloat16 with numpy)
import numpy as np
import orjson

import concourse.bacc as bacc
import concourse.bass as bass
import concourse.tile as tile
from concourse import mybir
from concourse.bass_utils import run_bass_kernel_spmd

N_CORES = 8
I_IN = 8
H = 33
HOR = 24
GW = 97  # gates psum width: z rows 0-32, memset rows 33-63, r rows 64-96
KC = H + I_IN  # 41: contraction dim of the concat matmuls

AF = mybir.ActivationFunctionType
DT = mybir.dt
BF16 = np.dtype("bfloat16")


# --------------------------------------------------------------------------
# walrus in this container rejects CTRL (Drain) instructions carrying more
# than one sync wait; Tile's kernel-tail drain always has several. Split
# them at the serialized-JSON level (mutating the live module corrupts it).
def _split_multiwait_drains(raw: bytes, max_waits: int = 1) -> bytes:
    m = orjson.loads(raw)
    changed = False
    for f in m["functions"]:
        for bb in f["blocks"]:
            out = []
            for inst in bb["instructions"]:
                si = inst.get("sync_info")
                ow = (si or {}).get("on_wait") or []
                if inst.get("opcode") == "Drain" and len(ow) > max_waits:
                    head, tail = ow[:-max_waits], ow[-max_waits:]
                    for k, w in enumerate(head):
                        clone = dict(inst)
                        clone["name"] = f"{inst['name']}-sw{k}"
                        clone["sync_info"] = {"on_update": [], "on_wait": [w]}
                        out.append(clone)
                    inst = dict(inst)
                    inst["sync_info"] = {
                        "on_update": si.get("on_update") or [],
                        "on_wait": tail,
                    }
                    changed = True
                out.append(inst)
            bb["instructions"] = out
    return orjson.dumps(m) if changed else raw


def _install_bir_patch(nc):
    orig = nc.to_json_bytes
    nc.to_json_bytes = lambda: _split_multiwait_drains(orig())


def build_gru_nc(B: int, T: int, finalize: bool = True, G: int = 3, repeat: int = 1,
                 elem16: bool = True):
    """Build the per-core Bass module (B = per-core batch). repeat>1 runs the
    whole recurrence multiple times (for wall-clock delta timing)."""
    nc = bacc.Bacc("TRN2", target_bir_lowering=False, debug=False)
    f32 = DT.float32
    edt = DT.float16 if elem16 else DT.float32

    # group column offsets (unequal groups allowed, e.g. 1024 = 342+341+341)
    goff = [round(g * B / G) for g in range(G + 1)]
    Bgs = [goff[g + 1] - goff[g] for g in range(G)]

    # host layout: xh[t, i, b] = x[b, t, i]  (fp16)
    xh = nc.dram_tensor("xh", [T, I_IN, B], edt, kind="ExternalInput").ap()
    # z/r gate lhsT [41, 33] each: rows 0-32 = W*_h (h-part), 33-40 = W*_x
    w_z = nc.dram_tensor("w_z", [KC, H], edt, kind="ExternalInput").ap()
    w_r = nc.dram_tensor("w_r", [KC, H], edt, kind="ExternalInput").ap()
    # m-correction lhsT [33, 97]: cols 0-32 = Wz_h, 33-63 zero, 64-96 = Wr_h
    w_gm = nc.dram_tensor("w_gm", [H, GW], edt, kind="ExternalInput").ap()
    # cand lhsT [41, 33]: rows 0-32 = Wh_h, rows 33-40 = Wh_x
    w_c = nc.dram_tensor("w_c", [KC, H], edt, kind="ExternalInput").ap()
    b_g = nc.dram_tensor("b_g", [GW, 1], f32, kind="ExternalInput").ap()
    b_h = nc.dram_tensor("b_h", [H, 1], f32, kind="ExternalInput").ap()
    w_fc = nc.dram_tensor("w_fc", [H, HOR], edt, kind="ExternalInput").ap()
    b_fc = nc.dram_tensor("b_fc", [HOR, 1], f32, kind="ExternalInput").ap()
    y = nc.dram_tensor("y", [HOR, B], f32, kind="ExternalOutput").ap()

    with tile.TileContext(nc) as tc:
        with ExitStack() as ctx:
            consts = ctx.enter_context(tc.tile_pool(name="consts", bufs=1))
            hpool = ctx.enter_context(tc.tile_pool(name="hpool", bufs=6))
            rhpool = ctx.enter_context(tc.tile_pool(name="rhpool", bufs=4))
            acts = ctx.enter_context(tc.tile_pool(name="acts", bufs=4))
            htpool = ctx.enter_context(tc.tile_pool(name="htpool", bufs=4))
            mpool = ctx.enter_context(tc.tile_pool(name="mpool", bufs=4))
            psum_g = ctx.enter_context(
                tc.tile_pool(name="psum_g", bufs=1, space="PSUM")
            )
            psum_c = ctx.enter_context(
                tc.tile_pool(name="psum_c", bufs=1, space="PSUM")
            )

            # ---- constants ----
            # z/r partial lhsT at base 64 (rhs = h tile rows 64-104);
            # m-correction lhsT at base 0 (rhs = m tile rows 0-32).
            wz_t = consts.tile([64 + KC, H], edt)  # rows 64-104 used
            wr_t = consts.tile([64 + KC, H], edt)
            wgm_t = consts.tile([GW, GW], edt)  # rows 64-96 used
            wc_t = consts.tile([KC, H], edt)
            bg_t = consts.tile([GW, 1], f32)
            bh_t = consts.tile([H, 1], f32)
            wfc_t = consts.tile([GW, HOR], edt)  # rows 64-96 used
            bfc_t = consts.tile([HOR, 1], f32)
            nc.sync.dma_start(wz_t[64 : 64 + KC, :], w_z[:])
            nc.sync.dma_start(wr_t[64 : 64 + KC, :], w_r[:])
            nc.sync.dma_start(wgm_t[64 : 64 + H, :], w_gm[:])
            nc.sync.dma_start(wc_t[:, :], w_c[:])
            nc.sync.dma_start(bg_t[:], b_g[:])
            nc.sync.dma_start(bh_t[:], b_h[:])
            nc.sync.dma_start(wfc_t[64 : 64 + H, :], w_fc[:])
            nc.sync.dma_start(bfc_t[:], b_fc[:])

            # ---- persistent psum tiles (one bank per group per kind);
            # gates rows 33-63 are never written by the mms but read by the
            # [97] sigmoid: memset once (mms only touch 0-32 / 64-96).
            # psum tiles are padded to 512 fp32 columns (one full bank) so
            # matmul outputs never cross a bank boundary; only [:, :Bg] used.
            P_g = []
            P_c = []
            for g in range(G):
                Bg = Bgs[g]
                P = psum_g.tile([GW, 512], f32, tag=f"pg{g}", name=f"pg{g}")
                nc.vector.memset(P[32:64, 0:Bg], 0.0)
                P_g.append(P)
                P_c.append(
                    psum_c.tile([H, 512], f32, tag=f"pc{g}", name=f"pc{g}")
                )

            # ---- per-group state: h rows 64-96, x rows 97-104.
            # Tile H_k holds h_k with x_{k+2} in its x rows (consumed by the
            # step-(k+2) partial matmul). Two zero-h init tiles seed the
            # pipeline: "H_{-2}" carries x_0, "H_{-1}" carries x_1.
            h_cur = [None] * G   # H_{t-1} while processing step t
            h_prev = [None] * G  # H_{t-2} while processing step t
            sig_cur = [None] * G
            m_cur = [None] * G

            def init_state(g, rep):
                Bg = Bgs[g]
                hm2 = hpool.tile(
                    [64 + KC, Bg], edt, tag=f"h{g}", name=f"h{g}_im2_{rep}"
                )
                nc.vector.memset(hm2[64 : 64 + H, :], 0.0)
                nc.sync.dma_start(
                    hm2[64 + H : 64 + KC, :], xh[0, :, goff[g] : goff[g + 1]]
                )
                hm1 = hpool.tile(
                    [64 + KC, Bg], edt, tag=f"h{g}", name=f"h{g}_im1_{rep}"
                )
                nc.vector.memset(hm1[64 : 64 + H, :], 0.0)
                nc.sync.dma_start(
                    hm1[64 + H : 64 + KC, :], xh[1, :, goff[g] : goff[g + 1]]
                )
                h_prev[g] = hm2
                h_cur[g] = hm1
                m0 = mpool.tile([GW, Bg], edt, tag=f"m{g}", name=f"m{g}_z{rep}")
                nc.vector.memset(m0[64 : 64 + H, :], 0.0)
                m_cur[g] = m0

            def emit_partial(g, t, rep):
                """Open the gates psum group for step t with the partial
                matmuls W_g.[h_{t-2}; x_t] (z and r, M=33 pairs). Runs off
                the chain as soon as sigma(t-1) has read the psum."""
                h = h_prev[g]
                P = P_g[g]
                Bg = Bgs[g]
                nc.tensor.matmul(
                    P[0:H, 0:Bg], wz_t[64 : 64 + KC, :], h[64 : 64 + KC, :],
                    start=True, stop=False, tile_position=(64, 0),
                )
                nc.tensor.matmul(
                    P[64 : 64 + H, 0:Bg], wr_t[64 : 64 + KC, :],
                    h[64 : 64 + KC, :],
                    start=True, stop=False, tile_position=(64, 64),
                    skip_group_check=True,
                )
                return P

            def emit_mcorr(g, t, rep):
                """Close the gates psum group for step t with the m_{t-1}
                correction W_gh.m (z/r M=33 stop pair)."""
                m = m_cur[g]
                P = P_g[g]
                Bg = Bgs[g]
                nc.tensor.matmul(
                    P[0:H, 0:Bg], wgm_t[64 : 64 + H, 0:H], m[64 : 64 + H, :],
                    start=False, stop=True, tile_position=(64, 0),
                )
                nc.tensor.matmul(
                    P[64 : 64 + H, 0:Bg], wgm_t[64 : 64 + H, 64 : 64 + H],
                    m[64 : 64 + H, :],
                    start=False, stop=True, tile_position=(64, 64),
                    skip_group_check=True,
                )

            def emit_front(g, t, rep):
                """sigma, rh, cand matmul (the chain head for step t)."""
                Bg = Bgs[g]
                h = h_cur[g]
                P = P_g[g]
                sig = acts.tile(
                    [GW, Bg], edt, tag=f"sig{g}", name=f"sig{g}_{rep}_{t}"
                )
                nc.scalar.activation(sig[:, :], P[:, 0:Bg], AF.Sigmoid, bias=bg_t[:])
                sig_cur[g] = sig
                # rh = r * h  (input bases 64,64) -> rows 0-32 of the rh tile
                rh = rhpool.tile([KC, Bg], edt, tag=f"rh{g}", name=f"rh{g}_{rep}_{t}")
                nc.sync.dma_start(rh[H:KC, :], xh[t, :, goff[g] : goff[g + 1]])
                nc.vector.tensor_mul(
                    rh[0:H, :], sig[64 : 64 + H, :], h[64 : 64 + H, :]
                )
                C = P_c[g]
                nc.tensor.matmul(
                    C[:, 0:Bg], wc_t[:, :], rh[:, :],
                    start=True, stop=True, tile_position=(0, 0),
                    skip_group_check=True,
                )
                return C

            def emit_back(g, t, rep):
                """tanh -> d = h~ - h -> m = z*d (adjacent DVE ops); the
                h' = h + m add runs off-chain on GpSimd/Pool."""
                Bg = Bgs[g]
                sig = sig_cur[g]
                h = h_cur[g]
                C = P_c[g]
                ht = htpool.tile(
                    [GW, Bg], edt, tag=f"ht{g}", name=f"ht{g}_{rep}_{t}"
                )
                nc.scalar.activation(
                    ht[64 : 64 + H, :], C[:, 0:Bg], AF.Tanh, bias=bh_t[:]
                )
                # d = h~ - h  (bases 64,64) -> rows 0-32 of the same tile
                nc.vector.tensor_sub(
                    ht[0:H, :], ht[64 : 64 + H, :], h[64 : 64 + H, :]
                )
                # m = z * d  (input bases 0,0); adjacent on DVE -> no sem gap
                m = mpool.tile([GW, Bg], edt, tag=f"m{g}", name=f"m{g}_{rep}_{t}")
                nc.vector.tensor_mul(m[64 : 64 + H, :], sig[0:H, :], ht[0:H, :])
                m_cur[g] = m

            def emit_hupd(g, t, rep):
                """h_t = h_{t-1} + m_t on Pool (off-chain), into a fresh
                tile whose x rows are filled with x_{t+2} by DMA."""
                Bg = Bgs[g]
                h = h_cur[g]
                m = m_cur[g]
                hn = hpool.tile(
                    [64 + KC, Bg], edt, tag=f"h{g}", name=f"h{g}_{rep}_{t}"
                )
                if t + 2 < T:
                    nc.gpsimd.dma_start(
                        hn[64 + H : 64 + KC, :],
                        xh[t + 2, :, goff[g] : goff[g + 1]],
                    )
                nc.gpsimd.tensor_add(
                    hn[64 : 64 + H, :], h[64 : 64 + H, :], m[64 : 64 + H, :]
                )
                h_prev[g] = h
                h_cur[g] = hn

            for rep in range(repeat):
                # Pipeline: at step t, group g's chain is
                #   [emit_mcorr(t)] -> emit_front(t) -> emit_back(t)
                # with emit_hupd(t) + emit_partial(t+1) trailing off-chain.
                # Step 0's m-correction uses a zero m tile (m_{-1}=0).
                for g in range(G):
                    init_state(g, rep)
                    emit_partial(g, 0, rep)
                def fr(g, t):
                    emit_mcorr(g, t, rep)
                    emit_front(g, t, rep)

                def bk(g, t):
                    if t < 0:
                        return
                    emit_back(g, t, rep)
                    emit_hupd(g, t, rep)
                    if t + 1 < T:
                        emit_partial(g, t + 1, rep)

                for t in range(T):
                    for i in range(G):
                        fr(i, t)
                        bk((i + G - 1) % G, t - 1 if i == 0 else t)
                bk(G - 1, T - 1)

            # ---- final FC ----
            for g in range(G):
                Bg = Bgs[g]
                pfc = psum_c.tile([HOR, 512], f32, tag=f"pc{g}", name=f"pfc{g}")
                nc.tensor.matmul(
                    pfc[:, 0:Bg], wfc_t[64 : 64 + H, :], h_cur[g][64 : 64 + H, :],
                    start=True, stop=True, tile_position=(64, 0),
                )
                y_sb = acts.tile([GW, Bg], f32, tag=f"sig{g}", name=f"ysb{g}")
                nc.scalar.activation(
                    y_sb[0:HOR, :], pfc[:, 0:Bg], AF.Identity, bias=bfc_t[:]
                )
                nc.sync.dma_start(y[:, goff[g] : goff[g + 1]], y_sb[0:HOR, :])

    if finalize:
        nc.finalize()
        _install_bir_patch(nc)
    return nc


def prep_weights(Wz, bz, Wr, br, Wh, bh, Wfc, bfc, elem16=True):
    ed = np.float16 if elem16 else np.float32

    def cat(w):
        out = np.zeros((KC, H), np.float32)
        out[0:H] = w[I_IN:]
        out[H:KC] = w[:I_IN]
        return out

    w_gm = np.zeros((H, GW), np.float32)
    w_gm[:, 0:H] = Wz[I_IN:]
    w_gm[:, 64 : 64 + H] = Wr[I_IN:]
    b_g = np.zeros((GW, 1), np.float32)
    b_g[0:H, 0] = bz
    b_g[64 : 64 + H, 0] = br
    return {
        "w_z": cat(Wz).astype(ed),
        "w_r": cat(Wr).astype(ed),
        "w_gm": w_gm.astype(ed),
        "w_c": cat(Wh).astype(ed),
        "b_g": b_g,
        "b_h": np.asarray(bh).reshape(H, 1).astype(np.float32),
        "w_fc": np.ascontiguousarray(Wfc).astype(ed),
        "b_fc": np.asarray(bfc).reshape(HOR, 1).astype(np.float32),
    }


def prep_x_one_core(xc, elem16=True):
    """xc [B, T, I] -> xh [T, I, B] fp16 for one core."""
    ed = np.float16 if elem16 else np.float32
    return np.ascontiguousarray(xc.transpose(1, 2, 0)).astype(ed)


def prep_inputs_one_core(x, Wz, bz, Wr, br, Wh, bh, Wfc, bfc, elem16=True):
    wmap = prep_weights(Wz, bz, Wr, br, Wh, bh, Wfc, bfc, elem16=elem16)
    return {"xh": prep_x_one_core(x, elem16=elem16), **wmap}


def run_gru(x, Wz, bz, Wr, br, Wh, bh, Wfc, bfc, n_cores=N_CORES, G=3,
            elem16=True, **spmd_kwargs):
    B_total, T, _ = x.shape
    B = B_total // n_cores
    nc = build_gru_nc(B, T, G=G, elem16=elem16)
    wmap = prep_weights(Wz, bz, Wr, br, Wh, bh, Wfc, bfc, elem16=elem16)
    in_maps = []
    for c in range(n_cores):
        xc = x[c * B : (c + 1) * B]  # [B, T, I]
        in_maps.append({"xh": prep_x_one_core(xc, elem16=elem16), **wmap})
    res = run_bass_kernel_spmd(
        nc, in_maps, core_ids=list(range(n_cores)), **spmd_kwargs
    )
    y = np.concatenate(
        [res.results[c]["y"].T for c in range(n_cores)], axis=0
    ).astype(np.float32)
    return y, res


def kernel(x, Wz, bz, Wr, br, Wh, bh, Wfc, bfc):
    y, _ = run_gru(x, Wz, bz, Wr, br, Wh, bh, Wfc, bfc)
    return y


# revision 15
# speedup vs baseline: 3.1485x; 2.1575x over previous
"""Trainium2 Bass kernel for the CustomGRU problem (v3, latency-optimized).

Reference semantics (fp32):
    z = sigmoid(x_t @ Wz_x + bz + h @ Wz_h)
    r = sigmoid(x_t @ Wr_x + br + h @ Wr_h)
    h~ = tanh(x_t @ Wh_x + bh + (r*h) @ Wh_h)
    h  = (1-z)*h + z*h~            (T=512 steps)
    out = h_T @ Wfc + bfc

Sharding: pure data parallel over batch (8192 -> 8 cores x 1024); the time
recurrence runs locally per core; the tiny weights are replicated.

The kernel is bound by the per-step serial chain latency (the recurrence),
not engine throughput, so v3 minimizes the chain:

    sigma -> rh -> mm_cand -> tanh -> d -> m -> mm_gm -> sigma(t+1)

  - deferred-accumulate gates matmul: h_t = h_{t-1} + m_t, so
    gates_pre(t+1) = [W_g.[h_{t-1}; x_{t+1}]]  (start mm, runs a full cycle
    early, off the chain) + W_gh.m_t (small stop mm right after m_t). The
    h' add itself is off-chain on GpSimd/Pool.
  - z and r are M=33 matmul pairs into one [97, Bg] psum (z rows 0-32,
    r rows 64-96; rows 33-63 memset once per psum buffer) -> ONE sigmoid
    ACT per step per group, biases ride the ACT bias operand.
  - x_t rides in the matmul rhs: h tiles are [105, Bg] with h at rows
    64-96 and x rows 97-104 (overwritten with x_{t+1} after the partial
    gates mm consumed x_t); the cand matmul is one K=41 instr on the
    rh tile (rh rows 0-32 + x rows 33-40).
  - d = h~ - h and m = z*d are adjacent DVE ops (no cross-engine hop);
    all tensor_tensor ops are fp16 SBUF dense (DVE 2x mode) with operand
    partition bases equal and 32-aligned (h at base 64, d at base 0).
  - G groups of Bg=B/G samples pipeline their chains to keep engines fed.
"""

import sys

sys.path.insert(0, "/opt/trn_rl_repo")

from contextlib import ExitStack

import ml_dtypes  # noqa: F401  (registers bfloat16 with numpy)
import numpy as np
import orjson

import concourse.bacc as bacc
import concourse.bass as bass
import concourse.tile as tile
from concourse import mybir
from concourse.bass_utils import run_bass_kernel_spmd

N_CORES = 8
I_IN = 8
H = 33
HOR = 24
GW = 97  # gates psum width: z rows 0-32, memset rows 33-63, r rows 64-96
KC = H + I_IN  # 41: contraction dim of the concat matmuls

AF = mybir.ActivationFunctionType
DT = mybir.dt
BF16 = np.dtype("bfloat16")


# --------------------------------------------------------------------------
# walrus in this container rejects CTRL (Drain) instructions carrying more
# than one sync wait; Tile's kernel-tail drain always has several. Split
# them at the serialized-JSON level (mutating the live module corrupts it).
def _split_multiwait_drains(raw: bytes, max_waits: int = 1) -> bytes:
    m = orjson.loads(raw)
    changed = False
    for f in m["functions"]:
        for bb in f["blocks"]:
            out = []
            for inst in bb["instructions"]:
                si = inst.get("sync_info")
                ow = (si or {}).get("on_wait") or []
                if inst.get("opcode") == "Drain" and len(ow) > max_waits:
                    head, tail = ow[:-max_waits], ow[-max_waits:]
                    for k, w in enumerate(head):
                        clone = dict(inst)
                        clone["name"] = f"{inst['name']}-sw{k}"
                        clone["sync_info"] = {"on_update": [], "on_wait": [w]}
                        out.append(clone)
                    inst = dict(inst)
                    inst["sync_info"] = {
                        "on_update": si.get("on_update") or [],
                        "on_wait": tail,
                    }
                    changed = True
                out.append(inst)
            bb["instructions"] = out
    return orjson.dumps(m) if changed else raw


def _install_bir_patch(nc):
    orig = nc.to_json_bytes
    nc.to_json_bytes = lambda: _split_multiwait_drains(orig())


def build_gru_nc(B: int, T: int, finalize: bool = True, G: int = 3, repeat: int = 1,
                 elem16: bool = True):
    """Build the per-core Bass module (B = per-core batch). repeat>1 runs the
    whole recurrence multiple times (for wall-clock delta timing)."""
    nc = bacc.Bacc("TRN2", target_bir_lowering=False, debug=False)
    f32 = DT.float32
    edt = DT.float16 if elem16 else DT.float32

    # group column offsets (unequal groups allowed, e.g. 1024 = 342+341+341)
    goff = [round(g * B / G) for g in range(G + 1)]
    Bgs = [goff[g + 1] - goff[g] for g in range(G)]

    # host layout: xh[t, i, b] = x[b, t, i]  (fp16)
    xh = nc.dram_tensor("xh", [T, I_IN, B], edt, kind="ExternalInput").ap()
    # gates lhsT [41, 97]: cols 0-32 = z, 33-63 zero, 64-96 = r; rows
    # 0-32 = W*_h (h-part), rows 33-40 = W*_x (x-part)
    w_g = nc.dram_tensor("w_g", [KC, GW], edt, kind="ExternalInput").ap()
    # m-correction lhsT [33, 97]: cols 0-32 = Wz_h, 33-63 zero, 64-96 = Wr_h
    w_gm = nc.dram_tensor("w_gm", [H, GW], edt, kind="ExternalInput").ap()
    # cand lhsT [41, 33]: rows 0-32 = Wh_h, rows 33-40 = Wh_x
    w_c = nc.dram_tensor("w_c", [KC, H], edt, kind="ExternalInput").ap()
    b_g = nc.dram_tensor("b_g", [GW, 1], f32, kind="ExternalInput").ap()
    b_h = nc.dram_tensor("b_h", [H, 1], f32, kind="ExternalInput").ap()
    w_fc = nc.dram_tensor("w_fc", [H, HOR], edt, kind="ExternalInput").ap()
    b_fc = nc.dram_tensor("b_fc", [HOR, 1], f32, kind="ExternalInput").ap()
    y = nc.dram_tensor("y", [HOR, B], f32, kind="ExternalOutput").ap()

    with tile.TileContext(nc) as tc:
        with ExitStack() as ctx:
            consts = ctx.enter_context(tc.tile_pool(name="consts", bufs=1))
            hpool = ctx.enter_context(tc.tile_pool(name="hpool", bufs=6))
            rhpool = ctx.enter_context(tc.tile_pool(name="rhpool", bufs=4))
            acts = ctx.enter_context(tc.tile_pool(name="acts", bufs=4))
            htpool = ctx.enter_context(tc.tile_pool(name="htpool", bufs=4))
            mpool = ctx.enter_context(tc.tile_pool(name="mpool", bufs=4))
            psum_g = ctx.enter_context(
                tc.tile_pool(name="psum_g", bufs=1, space="PSUM")
            )
            psum_c = ctx.enter_context(
                tc.tile_pool(name="psum_c", bufs=1, space="PSUM")
            )

            # ---- constants ----
            # z/r partial lhsT at base 64 (rhs = h tile rows 64-104);
            # m-correction lhsT at base 0 (rhs = m tile rows 0-32).
            wg_t = consts.tile([64 + KC, GW], edt)  # rows 64-104 used
            wgm_t = consts.tile([GW, GW], edt)  # rows 64-96 used
            wc_t = consts.tile([KC, H], edt)
            bg_t = consts.tile([GW, 1], f32)
            bh_t = consts.tile([H, 1], f32)
            wfc_t = consts.tile([GW, HOR], edt)  # rows 64-96 used
            bfc_t = consts.tile([HOR, 1], f32)
            nc.sync.dma_start(wg_t[64 : 64 + KC, :], w_g[:])
            nc.sync.dma_start(wgm_t[64 : 64 + H, :], w_gm[:])
            nc.sync.dma_start(wc_t[:, :], w_c[:])
            nc.sync.dma_start(bg_t[:], b_g[:])
            nc.sync.dma_start(bh_t[:], b_h[:])
            nc.sync.dma_start(wfc_t[64 : 64 + H, :], w_fc[:])
            nc.sync.dma_start(bfc_t[:], b_fc[:])

            # ---- persistent psum tiles (one bank per group per kind);
            # gates rows 33-63 are never written by the mms but read by the
            # [97] sigmoid: memset once (mms only touch 0-32 / 64-96).
            # psum tiles are padded to 512 fp32 columns (one full bank) so
            # matmul outputs never cross a bank boundary; only [:, :Bg] used.
            P_g = []
            P_c = []
            for g in range(G):
                Bg = Bgs[g]
                P = psum_g.tile([GW, 512], f32, tag=f"pg{g}", name=f"pg{g}")
                nc.vector.memset(P[32:64, 0:Bg], 0.0)
                P_g.append(P)
                P_c.append(
                    psum_c.tile([H, 512], f32, tag=f"pc{g}", name=f"pc{g}")
                )

            # ---- per-group state: h rows 64-96, x rows 97-104.
            # Tile H_k holds h_k with x_{k+2} in its x rows (consumed by the
            # step-(k+2) partial matmul). Two zero-h init tiles seed the
            # pipeline: "H_{-2}" carries x_0, "H_{-1}" carries x_1.
            h_cur = [None] * G   # H_{t-1} while processing step t
            h_prev = [None] * G  # H_{t-2} while processing step t
            sig_cur = [None] * G
            m_cur = [None] * G

            def init_state(g, rep):
                Bg = Bgs[g]
                hm2 = hpool.tile(
                    [64 + KC, Bg], edt, tag=f"h{g}", name=f"h{g}_im2_{rep}"
                )
                nc.vector.memset(hm2[64 : 64 + H, :], 0.0)
                nc.sync.dma_start(
                    hm2[64 + H : 64 + KC, :], xh[0, :, goff[g] : goff[g + 1]]
                )
                hm1 = hpool.tile(
                    [64 + KC, Bg], edt, tag=f"h{g}", name=f"h{g}_im1_{rep}"
                )
                nc.vector.memset(hm1[64 : 64 + H, :], 0.0)
                nc.sync.dma_start(
                    hm1[64 + H : 64 + KC, :], xh[1, :, goff[g] : goff[g + 1]]
                )
                h_prev[g] = hm2
                h_cur[g] = hm1
                m0 = mpool.tile([GW, Bg], edt, tag=f"m{g}", name=f"m{g}_z{rep}")
                nc.vector.memset(m0[64 : 64 + H, :], 0.0)
                m_cur[g] = m0

            def emit_partial(g, t, rep):
                """Open the gates psum group for step t with the partial
                matmuls W_g.[h_{t-2}; x_t] (z and r, M=33 pairs). Runs off
                the chain as soon as sigma(t-1) has read the psum."""
                h = h_prev[g]
                P = P_g[g]
                Bg = Bgs[g]
                nc.tensor.matmul(
                    P[0:GW, 0:Bg], wg_t[64 : 64 + KC, :], h[64 : 64 + KC, :],
                    start=True, stop=False, tile_position=(64, 0),
                )
                return P

            def emit_mcorr(g, t, rep):
                """Close the gates psum group for step t with the m_{t-1}
                correction W_gh.m (z/r M=33 stop pair)."""
                m = m_cur[g]
                P = P_g[g]
                Bg = Bgs[g]
                nc.tensor.matmul(
                    P[0:GW, 0:Bg], wgm_t[64 : 64 + H, :], m[64 : 64 + H, :],
                    start=False, stop=True, tile_position=(64, 0),
                )

            def emit_front(g, t, rep):
                """sigma, rh, cand matmul (the chain head for step t)."""
                Bg = Bgs[g]
                h = h_cur[g]
                P = P_g[g]
                sig = acts.tile(
                    [GW, Bg], edt, tag=f"sig{g}", name=f"sig{g}_{rep}_{t}"
                )
                nc.scalar.activation(sig[:, :], P[:, 0:Bg], AF.Sigmoid, bias=bg_t[:])
                sig_cur[g] = sig
                # rh = r * h  (input bases 64,64) -> rows 0-32 of the rh tile
                rh = rhpool.tile([KC, Bg], edt, tag=f"rh{g}", name=f"rh{g}_{rep}_{t}")
                nc.sync.dma_start(rh[H:KC, :], xh[t, :, goff[g] : goff[g + 1]])
                nc.vector.tensor_mul(
                    rh[0:H, :], sig[64 : 64 + H, :], h[64 : 64 + H, :]
                )
                C = P_c[g]
                nc.tensor.matmul(
                    C[:, 0:Bg], wc_t[:, :], rh[:, :],
                    start=True, stop=True, tile_position=(0, 0),
                    skip_group_check=True,
                )
                return C

            def emit_back(g, t, rep):
                """tanh -> d = h~ - h -> m = z*d (adjacent DVE ops); the
                h' = h + m add runs off-chain on GpSimd/Pool."""
                Bg = Bgs[g]
                sig = sig_cur[g]
                h = h_cur[g]
                C = P_c[g]
                ht = htpool.tile(
                    [GW, Bg], edt, tag=f"ht{g}", name=f"ht{g}_{rep}_{t}"
                )
                nc.scalar.activation(
                    ht[64 : 64 + H, :], C[:, 0:Bg], AF.Tanh, bias=bh_t[:]
                )
                # d = h~ - h  (bases 64,64) -> rows 0-32 of the same tile
                nc.vector.tensor_sub(
                    ht[0:H, :], ht[64 : 64 + H, :], h[64 : 64 + H, :]
                )
                # m = z * d  (input bases 0,0); adjacent on DVE -> no sem gap
                m = mpool.tile([GW, Bg], edt, tag=f"m{g}", name=f"m{g}_{rep}_{t}")
                nc.vector.tensor_mul(m[64 : 64 + H, :], sig[0:H, :], ht[0:H, :])
                m_cur[g] = m

            def emit_hupd(g, t, rep):
                """h_t = h_{t-1} + m_t on Pool (off-chain), into a fresh
                tile whose x rows are filled with x_{t+2} by DMA."""
                Bg = Bgs[g]
                h = h_cur[g]
                m = m_cur[g]
                hn = hpool.tile(
                    [64 + KC, Bg], edt, tag=f"h{g}", name=f"h{g}_{rep}_{t}"
                )
                if t + 2 < T:
                    nc.gpsimd.dma_start(
                        hn[64 + H : 64 + KC, :],
                        xh[t + 2, :, goff[g] : goff[g + 1]],
                    )
                eng = nc.gpsimd if g == G - 1 else nc.vector
                eng.tensor_add(
                    hn[64 : 64 + H, :], h[64 : 64 + H, :], m[64 : 64 + H, :]
                )
                h_prev[g] = h
                h_cur[g] = hn

            for rep in range(repeat):
                # Pipeline: at step t, group g's chain is
                #   [emit_mcorr(t)] -> emit_front(t) -> emit_back(t)
                # with emit_hupd(t) + emit_partial(t+1) trailing off-chain.
                # Step 0's m-correction uses a zero m tile (m_{-1}=0).
                for g in range(G):
                    init_state(g, rep)
                    emit_partial(g, 0, rep)
                def fr(g, t):
                    if t == 0:
                        emit_mcorr(g, 0, rep)
                    emit_front(g, t, rep)

                def bk(g, t):
                    if t < 0:
                        return
                    emit_back(g, t, rep)
                    emit_hupd(g, t, rep)
                    if t + 1 < T:
                        emit_partial(g, t + 1, rep)
                        # m_cur[g] is m_t here: the step-(t+1) correction can
                        # queue right behind its partials on the PE
                        emit_mcorr(g, t + 1, rep)

                for t in range(T):
                    for i in range(G):
                        fr(i, t)
                        bk((i + G - 1) % G, t - 1 if i == 0 else t)
                bk(G - 1, T - 1)

            # ---- final FC ----
            for g in range(G):
                Bg = Bgs[g]
                pfc = psum_c.tile([HOR, 512], f32, tag=f"pc{g}", name=f"pfc{g}")
                nc.tensor.matmul(
                    pfc[:, 0:Bg], wfc_t[64 : 64 + H, :], h_cur[g][64 : 64 + H, :],
                    start=True, stop=True, tile_position=(64, 0),
                )
                y_sb = acts.tile([GW, Bg], f32, tag=f"sig{g}", name=f"ysb{g}")
                nc.scalar.activation(
                    y_sb[0:HOR, :], pfc[:, 0:Bg], AF.Identity, bias=bfc_t[:]
                )
                nc.sync.dma_start(y[:, goff[g] : goff[g + 1]], y_sb[0:HOR, :])

    if finalize:
        nc.finalize()
        _install_bir_patch(nc)
    return nc


def prep_weights(Wz, bz, Wr, br, Wh, bh, Wfc, bfc, elem16=True):
    ed = np.float16 if elem16 else np.float32

    def cat(w):
        out = np.zeros((KC, H), np.float32)
        out[0:H] = w[I_IN:]
        out[H:KC] = w[:I_IN]
        return out

    w_g = np.zeros((KC, GW), np.float32)
    w_g[0:H, 0:H] = Wz[I_IN:]
    w_g[H:KC, 0:H] = Wz[:I_IN]
    w_g[0:H, 64 : 64 + H] = Wr[I_IN:]
    w_g[H:KC, 64 : 64 + H] = Wr[:I_IN]
    w_gm = np.zeros((H, GW), np.float32)
    w_gm[:, 0:H] = Wz[I_IN:]
    w_gm[:, 64 : 64 + H] = Wr[I_IN:]
    b_g = np.zeros((GW, 1), np.float32)
    b_g[0:H, 0] = bz
    b_g[64 : 64 + H, 0] = br
    return {
        "w_g": w_g.astype(ed),
        "w_gm": w_gm.astype(ed),
        "w_c": cat(Wh).astype(ed),
        "b_g": b_g,
        "b_h": np.asarray(bh).reshape(H, 1).astype(np.float32),
        "w_fc": np.ascontiguousarray(Wfc).astype(ed),
        "b_fc": np.asarray(bfc).reshape(HOR, 1).astype(np.float32),
    }


def prep_x_one_core(xc, elem16=True):
    """xc [B, T, I] -> xh [T, I, B] fp16 for one core."""
    ed = np.float16 if elem16 else np.float32
    return np.ascontiguousarray(xc.transpose(1, 2, 0)).astype(ed)


def prep_inputs_one_core(x, Wz, bz, Wr, br, Wh, bh, Wfc, bfc, elem16=True):
    wmap = prep_weights(Wz, bz, Wr, br, Wh, bh, Wfc, bfc, elem16=elem16)
    return {"xh": prep_x_one_core(x, elem16=elem16), **wmap}


def run_gru(x, Wz, bz, Wr, br, Wh, bh, Wfc, bfc, n_cores=N_CORES, G=3,
            elem16=True, **spmd_kwargs):
    B_total, T, _ = x.shape
    B = B_total // n_cores
    nc = build_gru_nc(B, T, G=G, elem16=elem16)
    wmap = prep_weights(Wz, bz, Wr, br, Wh, bh, Wfc, bfc, elem16=elem16)
    in_maps = []
    for c in range(n_cores):
        xc = x[c * B : (c + 1) * B]  # [B, T, I]
        in_maps.append({"xh": prep_x_one_core(xc, elem16=elem16), **wmap})
    res = run_bass_kernel_spmd(
        nc, in_maps, core_ids=list(range(n_cores)), **spmd_kwargs
    )
    y = np.concatenate(
        [res.results[c]["y"].T for c in range(n_cores)], axis=0
    ).astype(np.float32)
    return y, res


def kernel(x, Wz, bz, Wr, br, Wh, bh, Wfc, bfc):
    y, _ = run_gru(x, Wz, bz, Wr, br, Wh, bh, Wfc, bfc)
    return y
